# revision 17
# baseline (speedup 1.0000x reference)
"""Trainium2 Bass kernel for nn_MixtureLinear.

Math:  out[b,n,d] = sum_{c,r} input[b,n,c] * weight[d,c,r] * coef[n,r]
                    + sum_r coef[n,r] * bias[d,r]

Sharding: data-parallel over batch (B == 8 == n_cores).

Per-core formulation: ONE fused matmul with contraction K = C*R = 8192 by
folding coef into the activations:
    xp[r*C+c, n] = input[b,n,c] * coef[n,r]      (lhsT)
    wt[r*C+c, d] = weight[d,c,r]                 (rhs, streamed from HBM)
    out[n, d]    = xp.T @ wt + coef @ bias.T
Each [128n x 512d] output tile accumulates the full contraction in one PSUM
bank with no intermediate drains. The last J k-tiles run as fp8-e4m3
DoubleRow matmuls (2 k-planes per instruction, ~2x MAC rate) with
product-preserving scaling xp/8, wt*8 — exact-data numpy study: rel err
0.0138 at J=4 vs the 2e-2 gate. The bias term (host-precomputed
coef @ bias.T, bf16) is added by the DVE during each PSUM drain.

xp is generated ON DEVICE by the DVE (xt tile x broadcast coef row,
~0.7us per [128,1024] tile vs 1.73us PE consumption) into a rolling pool,
regenerated for each d-half. This cuts DMA-in from ~37MB (host-folded xp
was 15.7MB/core) to ~21.5MB — below the ~300GB/s aggregate channel
capacity that saturated and stalled the PE in earlier versions.

Schedule notes (from perfetto traces):
- ~6us fixed preamble; DMA transfers land ~2.5-3us after issue; each
  dma_start costs ~0.6us of issue time on its queue (sync/scalar/gpsimd
  loaded in parallel, first-use tiles issued first).
- F=64 dummy matmuls on a memset tile hold the PE p-state ramp (~3us to
  full speed, resets on idle) until the first real operands are ready.
- Both d-halves close each output tile early (m-major tail) so DVE drains
  (+bias add) and stores pipeline against the remaining matmuls; the
  d-half handoff reuses the same 8 PSUM banks with zero PE stall. dt1's
  first xp generations are emitted BEFORE dt0's drains on the vector
  queue so the PE never waits on a generation at the handoff.
- gpsimd's end-of-queue DRAIN costs ~8us; its last issue is mid-kernel
  (dt0 stores) so that drain overlaps compute instead of the exit barrier.
"""

import sys

if "/opt/trn_rl_repo" not in sys.path:
    sys.path.insert(0, "/opt/trn_rl_repo")

import numpy as np

B, N, C, D, R = 8, 1024, 1024, 1024, 8
P = 128        # SBUF partitions
DTILE = 512    # matmul moving free dim (one fp32 PSUM bank)
K = C * R      # fused contraction
KT = K // P    # 64 contraction tiles
MT = N // P    # 8 token tiles
CT = C // P    # 8 xt tiles
DT = D // DTILE  # 2 output column tiles
N_CORES = 8
NDUMMY = 48    # warmup matmuls (F=64, ~64ns each) ramping PE during DMA wait
J = 4          # k-tiles (of KT) computed in fp8 DoubleRow; even, >= 0
KB = KT - J    # bf16 k-tiles
JD = J // 2    # DoubleRow instructions per (m, dt)
MTAIL = 2      # bf16 k-rows folded into each d-half's m-major tail
XPB = 14       # rolling xp pool depth
GEN_AHEAD = 6  # dt1 xp generations emitted before dt0's drains

_CACHE = {}


def _build_nc():
    import concourse.mybir as mybir
    import concourse.tile as tile
    from concourse import bacc

    f32 = mybir.dt.float32
    bf16 = mybir.dt.bfloat16
    fp8 = mybir.dt.float8e4
    mult = mybir.AluOpType.mult
    add = mybir.AluOpType.add
    DR = mybir.MatmulPerfMode.DoubleRow

    nc = bacc.Bacc()
    xt = nc.dram_tensor("xt", [C, N], bf16, kind="ExternalInput")
    xp01 = nc.dram_tensor("xp01", [2 * P, N], bf16, kind="ExternalInput")
    coefbc = nc.dram_tensor("coefbc", [R * P, N], bf16, kind="ExternalInput")
    wt = nc.dram_tensor("wt", [KB * P, D], bf16, kind="ExternalInput")
    biasnd = nc.dram_tensor("biasnd", [N, D], bf16, kind="ExternalInput")
    out = nc.dram_tensor("out", [N, D], f32, kind="ExternalOutput")
    if J:
        wt8 = nc.dram_tensor("wt8", [DT * JD * P, 2 * DTILE], fp8, kind="ExternalInput")

    with tile.TileContext(nc) as tc:
        with (
            tc.tile_pool(name="consts", bufs=1) as cpool,
            tc.tile_pool(name="wpool", bufs=28) as wpool,
            tc.tile_pool(name="w8pool", bufs=4) as w8pool,
            tc.tile_pool(name="xppool", bufs=XPB) as xppool,
            tc.tile_pool(name="stpool", bufs=6) as stpool,
            tc.tile_pool(name="psum", bufs=1, space="PSUM") as pspool,
        ):
            ps = [
                pspool.tile([P, DTILE], f32, name=f"ps{m}", tag=f"ps{m}", bufs=1)
                for m in range(MT)
            ]

            # warmup: PE ramp fodder with no DMA dependency
            warm = cpool.tile([P, 64], bf16, name="warm", tag="warm")
            nc.vector.memset(warm, 0.0)
            for _ in range(NDUMMY):
                nc.tensor.matmul(
                    ps[0][0:64, 0:64], warm, warm[:, 0:64], start=True, stop=True
                )
            for _ in range(12):
                nc.tensor.matmul(
                    ps[0][0:64, 0:16], warm, warm[:, 0:16], start=True, stop=True
                )

            # --- DMA issue streams (issue cost ~0.6us each; 3 queues) ---
            # gpsimd: host-folded xp tiles k=0,1 first (they gate the first
            # matmuls and skip the DVE-generation dependency), then xt tiles
            xp01_sb = [
                cpool.tile([P, N], bf16, name=f"xp01_{k}", tag=f"xp01_{k}")
                for k in range(2)
            ]
            nc.gpsimd.dma_start(xp01_sb[0][:, 0:512], xp01[0:P, 0:512])
            nc.gpsimd.dma_start(xp01_sb[0][:, 512:1024], xp01[0:P, 512:1024])
            nc.gpsimd.dma_start(xp01_sb[1], xp01[P : 2 * P, :])
            xt_sb = [
                cpool.tile([P, N], bf16, name=f"xt{c}", tag=f"xt{c}")
                for c in range(CT)
            ]
            for c in range(CT):
                nc.gpsimd.dma_start(xt_sb[c], xt[c * P : (c + 1) * P, :])

            # scalar: early coef-broadcast rows. cb[r] is first used by the
            # k = r*8 generation at ~(11 + 13.8r)us, so cb0..3 load here
            # while cb4..7 ride the sync stream after the bias tiles.
            cb_sb = []
            for r in range(R):
                cb_sb.append(cpool.tile([P, N], bf16, name=f"cb{r}", tag=f"cb{r}"))
            nc.scalar.dma_start(cb_sb[0][:, 0:512], coefbc[0:P, 0:512])
            nc.scalar.dma_start(cb_sb[0][:, 512:1024], coefbc[0:P, 512:1024])
            for r in range(1, 4):
                nc.scalar.dma_start(cb_sb[r], coefbc[r * P : (r + 1) * P, :])

            # sync: wt stream in consumption order, with the bias tiles
            # slotted in after wt[8,0] (past the startup crunch, well before
            # their first use at the dt0 drains)
            wt_sb = {}
            bias_sb = {}

            def load_wt(k, dt):
                if k < KB:
                    w = wpool.tile([P, DTILE], bf16, name="w", tag="w")
                    nc.sync.dma_start(
                        w, wt[k * P : (k + 1) * P, dt * DTILE : (dt + 1) * DTILE]
                    )
                else:
                    kk = k - KB
                    w = w8pool.tile([P, 2, DTILE], fp8, name="w8", tag="w8")
                    nc.sync.dma_start(
                        w, wt8[(dt * JD + kk) * P : (dt * JD + kk + 1) * P, :]
                    )
                wt_sb[k, dt] = w

            NK = KB + JD  # matmul instructions per (m, dt) chain
            # head of the wt stream first; then the non-urgent residents
            # (bias tiles ~117us out, cb rows ~40us out) slot in behind it,
            # keeping the startup window under the DMA channel capacity
            for k in range(26):
                load_wt(k, 0)
            for dt in range(DT):
                for m in range(MT):
                    t = cpool.tile([P, DTILE], bf16, name=f"b{dt}_{m}", tag=f"b{dt}_{m}")
                    nc.sync.dma_start(
                        t, biasnd[m * P : (m + 1) * P, dt * DTILE : (dt + 1) * DTILE]
                    )
                    bias_sb[dt, m] = t
            for r in range(4, R):
                nc.sync.dma_start(cb_sb[r], coefbc[r * P : (r + 1) * P, :])
            for dt in range(DT):
                for k in range(NK):
                    if (k, dt) not in wt_sb:
                        load_wt(k, dt)

            # --- xp generation (vector + gpsimd, ~1.2us per tile on DVE
            # vs 1.73us PE consumption per tile) ---
            xps = {}

            def gen_xp(dt, k, split=1, eng=None):
                # xp[k] = xt[c-tile] * coef-broadcast[r],  k = r*CT + c-tile
                t = xppool.tile([P, N], bf16, name="xp", tag="xp")
                r, c = k // CT, k % CT
                fw = N // split
                for s in range(split):
                    fsl = slice(s * fw, (s + 1) * fw)
                    (eng or nc.vector).scalar_tensor_tensor(
                        t[:, fsl], xt_sb[c][:, fsl], 1.0, cb_sb[r][:, fsl], mult, mult
                    )
                xps[dt, k] = t

            # fp8 DoubleRow lhsT tiles: generated once, resident, scale 1/8
            # folded into the DVE op. Plane i of tile kk covers fused-k rows
            # (KB + 2*kk + i)*P .. +P  ==  (r=R-1, c-tiles CT-J+2*kk+i).
            xp8_sb = []
            for kk in range(JD):
                t = cpool.tile([P, 2, N], fp8, name=f"xp8_{kk}", tag=f"xp8_{kk}")
                xp8_sb.append(t)

            def gen_xp8(kk, i):
                c = (KB + 2 * kk + i) % CT
                r = (KB + 2 * kk + i) // CT
                nc.vector.scalar_tensor_tensor(
                    xp8_sb[kk][:, i, :], xt_sb[c], 0.125, cb_sb[r], mult, mult
                )

            def mm(m, k, dt, start, stop):
                if k < KB:
                    nc.tensor.matmul(
                        ps[m],
                        xps[dt, k][:, m * P : (m + 1) * P],
                        wt_sb[k, dt],
                        start=start,
                        stop=stop,
                    )
                else:
                    nc.tensor.matmul(
                        ps[m],
                        xp8_sb[k - KB][:, :, m * P : (m + 1) * P],
                        wt_sb[k, dt],
                        start=start,
                        stop=stop,
                        perf_mode=DR,
                    )

            # dt0 generations, all on vector (the STT opcode only exists on
            # the DVE): 1.22us/tile vs 1.73us PE consumption, pool-paced ~14
            # tiles ahead. The fp8 tiles (first used at dt0's tail, ~115us)
            # are generated around k=40 where the lookahead buffer absorbs
            # their ~5us cost without stalling the bf16 stream.
            for dt in range(DT):
                xps[dt, 0], xps[dt, 1] = xp01_sb[0], xp01_sb[1]
            for k in range(2, KB):
                if k == 40:
                    for kk in range(JD):
                        for i in range(2):
                            gen_xp8(kk, i)
                gen_xp(0, k)

            NTAIL = MTAIL + JD
            for dt in range(DT):
                dsl = slice(dt * DTILE, (dt + 1) * DTILE)
                for k in range(NK - NTAIL):
                    for m in range(MT):
                        mm(m, k, dt, k == 0, False)
                if dt == 0:
                    # emit dt1's first generations ahead of dt0's drains on
                    # the vector queue so the handoff never waits on a gen
                    for k in range(2, GEN_AHEAD):
                        gen_xp(1, k)
                # m-major tail: close each bank early so the DVE drain (+bias
                # add) and store pipeline against the remaining matmuls
                for m in range(MT):
                    for k in range(NK - NTAIL, NK):
                        mm(m, k, dt, False, k == NK - 1)
                    stage = stpool.tile([P, DTILE], f32, name="st", tag="st")
                    nc.vector.scalar_tensor_tensor(
                        stage, ps[m], 1.0, bias_sb[dt, m], mult, add
                    )
                    if dt < DT - 1:
                        # mid-kernel stores: gpsimd (its ~8us end-drain then
                        # overlaps compute, not the exit barrier)
                        nc.gpsimd.dma_start(out[m * P : (m + 1) * P, dsl], stage)
                    else:
                        splits = 2 if m >= MT - 2 else 1
                        engs = [nc.sync, nc.scalar]
                        rw = P // splits
                        for s in range(splits):
                            engs[(m + s) % 2].dma_start(
                                out[m * P + s * rw : m * P + (s + 1) * rw, dsl],
                                stage[s * rw : (s + 1) * rw, :],
                            )
                if dt == 0:
                    for k in range(GEN_AHEAD, KB):
                        gen_xp(1, k)
    nc.finalize()
    return nc


def _get_nc():
    if "nc" not in _CACHE:
        _CACHE["nc"] = _build_nc()
    return _CACHE["nc"]


def _prepare_in_maps(inputs):
    import ml_dtypes

    bf = ml_dtypes.bfloat16
    f8 = ml_dtypes.float8_e4m3fn
    f32 = np.float32
    input_ = np.asarray(inputs["input"], dtype=f32)
    weight = np.asarray(inputs["weight"], dtype=f32)
    bias = np.asarray(inputs["bias"], dtype=f32)
    coef = np.asarray(inputs["coef"], dtype=f32)

    wt_full = np.ascontiguousarray(weight.transpose(2, 1, 0)).reshape(K, D)
    wt = np.ascontiguousarray(wt_full[: KB * P]).astype(bf)
    biasnd = np.ascontiguousarray(coef @ bias.T).astype(bf)
    # coef rows broadcast across partitions: coefbc[r*P+p, n] = coef[n, r]
    coefbc = np.ascontiguousarray(
        np.broadcast_to(coef.T[:, None, :], (R, P, N)).reshape(R * P, N)
    ).astype(bf)

    shared = {"wt": wt, "biasnd": biasnd, "coefbc": coefbc}
    if J:
        w8 = (wt_full[KB * P :] * 8.0).astype(f8)  # [J*P, D]
        # [dt, kk, p, i, f] -> rows (dt*JD+kk)*P+p, cols i*DTILE+f
        w8r = w8.reshape(JD, 2, P, DT, DTILE)
        shared["wt8"] = np.ascontiguousarray(
            w8r.transpose(3, 0, 2, 1, 4).reshape(DT * JD * P, 2 * DTILE)
        )

    coefT_f32 = coef.T
    in_maps = []
    for b in range(B):
        xt_b = input_[b].T  # [C, N]
        xp01 = (coefT_f32[0][None, :] * xt_b[: 2 * P]).astype(bf)
        m = {
            "xt": np.ascontiguousarray(xt_b).astype(bf),
            "xp01": np.ascontiguousarray(xp01),
            **shared,
        }
        in_maps.append(m)
    return in_maps


def _install_ntff_hook_shim():
    """The agent image lacks antenv.axon_hooks; recreate it from the ctypes
    hook factory in trn_agent_boot so trace=True can capture NTFF profiles."""
    import types

    if "antenv.axon_hooks" in sys.modules:
        return
    try:
        from trn_agent_boot.trn_boot import _ntff_profile_via_ctypes

        hook = _ntff_profile_via_ctypes("/opt/axon/libaxon_pjrt.so")
        mod = types.ModuleType("antenv.axon_hooks")
        mod.get_axon_ntff_profile_hook = lambda: hook
        sys.modules["antenv.axon_hooks"] = mod
    except Exception as e:  # profiling is best-effort; execution still works
        print(f"ntff hook shim unavailable: {e}")


def _run(inputs, trace=False, **kwargs):
    from concourse.bass_utils import run_bass_kernel_spmd

    if trace:
        _install_ntff_hook_shim()
    in_maps = _prepare_in_maps(inputs)
    nc = _get_nc()
    res = run_bass_kernel_spmd(
        nc, in_maps, core_ids=list(range(N_CORES)), trace=trace, **kwargs
    )
    out = np.stack([r["out"] for r in res.results], axis=0)
    return out, res


def kernel(**inputs) -> np.ndarray:
    out, _ = _run(inputs)
    return out


# revision 18
# speedup vs baseline: 1.0230x; 1.0230x over previous
"""Trainium2 Bass kernel for nn_MixtureLinear.

Math:  out[b,n,d] = sum_{c,r} input[b,n,c] * weight[d,c,r] * coef[n,r]
                    + sum_r coef[n,r] * bias[d,r]

Sharding: data-parallel over batch (B == 8 == n_cores).

Per-core formulation: ONE fused matmul with contraction K = C*R = 8192 by
folding coef into the activations:
    xp[r*C+c, n] = input[b,n,c] * coef[n,r]      (lhsT)
    wt[r*C+c, d] = weight[d,c,r]                 (rhs, streamed from HBM)
    out[n, d]    = xp.T @ wt + coef @ bias.T
Each [128n x 512d] output tile accumulates the full contraction in one PSUM
bank with no intermediate drains. The last J k-tiles run as fp8-e4m3
DoubleRow matmuls (2 k-planes per instruction, ~2x MAC rate) with
product-preserving scaling xp/8, wt*8 — exact-data numpy study: rel err
0.0138 at J=4 vs the 2e-2 gate. The bias term (host-precomputed
coef @ bias.T, bf16) is added by the DVE during each PSUM drain.

xp is generated ON DEVICE by the DVE (xt tile x broadcast coef row,
~0.7us per [128,1024] tile vs 1.73us PE consumption) into a rolling pool,
regenerated for each d-half. This cuts DMA-in from ~37MB (host-folded xp
was 15.7MB/core) to ~21.5MB — below the ~300GB/s aggregate channel
capacity that saturated and stalled the PE in earlier versions.

Schedule notes (from perfetto traces):
- ~6us fixed preamble; DMA transfers land ~2.5-3us after issue; each
  dma_start costs ~0.6us of issue time on its queue (sync/scalar/gpsimd
  loaded in parallel, first-use tiles issued first).
- F=64 dummy matmuls on a memset tile hold the PE p-state ramp (~3us to
  full speed, resets on idle) until the first real operands are ready.
- Both d-halves close each output tile early (m-major tail) so DVE drains
  (+bias add) and stores pipeline against the remaining matmuls; the
  d-half handoff reuses the same 8 PSUM banks with zero PE stall. dt1's
  first xp generations are emitted BEFORE dt0's drains on the vector
  queue so the PE never waits on a generation at the handoff.
- gpsimd's end-of-queue DRAIN costs ~8us; its last issue is mid-kernel
  (dt0 stores) so that drain overlaps compute instead of the exit barrier.
"""

import sys

if "/opt/trn_rl_repo" not in sys.path:
    sys.path.insert(0, "/opt/trn_rl_repo")

import numpy as np

B, N, C, D, R = 8, 1024, 1024, 1024, 8
P = 128        # SBUF partitions
DTILE = 512    # matmul moving free dim (one fp32 PSUM bank)
K = C * R      # fused contraction
KT = K // P    # 64 contraction tiles
MT = N // P    # 8 token tiles
CT = C // P    # 8 xt tiles
DT = D // DTILE  # 2 output column tiles
N_CORES = 8
NDUMMY = 48    # warmup matmuls (F=64, ~64ns each) ramping PE during DMA wait
J = 4          # k-tiles (of KT) computed in fp8 DoubleRow; even, >= 0
KB = KT - J    # bf16 k-tiles
JD = J // 2    # DoubleRow instructions per (m, dt)
MTAIL = 2      # bf16 k-rows folded into each d-half's m-major tail
XPB = 14       # rolling xp pool depth
GEN_AHEAD = 6  # dt1 xp generations emitted before dt0's drains

_CACHE = {}


def _build_nc():
    import concourse.mybir as mybir
    import concourse.tile as tile
    from concourse import bacc

    f32 = mybir.dt.float32
    bf16 = mybir.dt.bfloat16
    fp8 = mybir.dt.float8e4
    mult = mybir.AluOpType.mult
    add = mybir.AluOpType.add
    DR = mybir.MatmulPerfMode.DoubleRow

    nc = bacc.Bacc()
    xt = nc.dram_tensor("xt", [C, N], bf16, kind="ExternalInput")
    xp01 = nc.dram_tensor("xp01", [2 * P, N], bf16, kind="ExternalInput")
    coefbc = nc.dram_tensor("coefbc", [R * P, N], bf16, kind="ExternalInput")
    wt = nc.dram_tensor("wt", [KB * P, D], bf16, kind="ExternalInput")
    biasnd = nc.dram_tensor("biasnd", [N, D], bf16, kind="ExternalInput")
    out = nc.dram_tensor("out", [N, D], f32, kind="ExternalOutput")
    if J:
        wt8 = nc.dram_tensor("wt8", [DT * JD * P, 2 * DTILE], fp8, kind="ExternalInput")

    with tile.TileContext(nc) as tc:
        with (
            tc.tile_pool(name="consts", bufs=1) as cpool,
            tc.tile_pool(name="wpool", bufs=28) as wpool,
            tc.tile_pool(name="w8pool", bufs=4) as w8pool,
            tc.tile_pool(name="xppool", bufs=XPB) as xppool,
            tc.tile_pool(name="stpool", bufs=6) as stpool,
            tc.tile_pool(name="psum", bufs=1, space="PSUM") as pspool,
        ):
            ps = [
                pspool.tile([P, DTILE], f32, name=f"ps{m}", tag=f"ps{m}", bufs=1)
                for m in range(MT)
            ]

            # warmup: PE ramp fodder with no DMA dependency
            warm = cpool.tile([P, 64], bf16, name="warm", tag="warm")
            nc.vector.memset(warm, 0.0)
            for _ in range(NDUMMY):
                nc.tensor.matmul(
                    ps[0][0:64, 0:64], warm, warm[:, 0:64], start=True, stop=True
                )
            for _ in range(12):
                nc.tensor.matmul(
                    ps[0][0:64, 0:16], warm, warm[:, 0:16], start=True, stop=True
                )

            # --- DMA issue streams (issue cost ~0.6us each; 3 queues) ---
            # gpsimd: host-folded xp tiles k=0,1 first (they gate the first
            # matmuls and skip the DVE-generation dependency), then xt tiles
            xp01_sb = [
                cpool.tile([P, N], bf16, name=f"xp01_{k}", tag=f"xp01_{k}")
                for k in range(2)
            ]
            nc.gpsimd.dma_start(xp01_sb[0][:, 0:512], xp01[0:P, 0:512])
            nc.gpsimd.dma_start(xp01_sb[0][:, 512:1024], xp01[0:P, 512:1024])
            nc.gpsimd.dma_start(xp01_sb[1], xp01[P : 2 * P, :])
            xt_sb = [
                cpool.tile([P, N], bf16, name=f"xt{c}", tag=f"xt{c}")
                for c in range(CT)
            ]
            # xt2..7 first: the DVE generations start at k=2 (k=0,1 are the
            # host-folded xp01 tiles); xt0/xt1 are first used at k=8,9 (~25us)
            for c in list(range(2, CT)) + [0, 1]:
                nc.gpsimd.dma_start(xt_sb[c], xt[c * P : (c + 1) * P, :])

            # scalar: early coef-broadcast rows. cb[r] is first used by the
            # k = r*8 generation at ~(11 + 13.8r)us, so cb0..3 load here
            # while cb4..7 ride the sync stream after the bias tiles.
            cb_sb = []
            for r in range(R):
                cb_sb.append(cpool.tile([P, N], bf16, name=f"cb{r}", tag=f"cb{r}"))
            nc.scalar.dma_start(cb_sb[0][:, 0:512], coefbc[0:P, 0:512])
            nc.scalar.dma_start(cb_sb[0][:, 512:1024], coefbc[0:P, 512:1024])
            for r in range(1, 4):
                nc.scalar.dma_start(cb_sb[r], coefbc[r * P : (r + 1) * P, :])

            # sync: wt stream in consumption order, with the bias tiles
            # slotted in after wt[8,0] (past the startup crunch, well before
            # their first use at the dt0 drains)
            wt_sb = {}
            bias_sb = {}

            def load_wt(k, dt):
                if k < KB:
                    w = wpool.tile([P, DTILE], bf16, name="w", tag="w")
                    nc.sync.dma_start(
                        w, wt[k * P : (k + 1) * P, dt * DTILE : (dt + 1) * DTILE]
                    )
                else:
                    kk = k - KB
                    w = w8pool.tile([P, 2, DTILE], fp8, name="w8", tag="w8")
                    nc.sync.dma_start(
                        w, wt8[(dt * JD + kk) * P : (dt * JD + kk + 1) * P, :]
                    )
                wt_sb[k, dt] = w

            NK = KB + JD  # matmul instructions per (m, dt) chain
            # head of the wt stream first; then the non-urgent residents
            # (bias tiles ~117us out, cb rows ~40us out) slot in behind it,
            # keeping the startup window under the DMA channel capacity
            for k in range(26):
                load_wt(k, 0)
            for dt in range(DT):
                for m in range(MT):
                    t = cpool.tile([P, DTILE], bf16, name=f"b{dt}_{m}", tag=f"b{dt}_{m}")
                    nc.sync.dma_start(
                        t, biasnd[m * P : (m + 1) * P, dt * DTILE : (dt + 1) * DTILE]
                    )
                    bias_sb[dt, m] = t
            for r in range(4, R):
                nc.sync.dma_start(cb_sb[r], coefbc[r * P : (r + 1) * P, :])
            for dt in range(DT):
                for k in range(NK):
                    if (k, dt) not in wt_sb:
                        load_wt(k, dt)

            # --- xp generation (vector + gpsimd, ~1.2us per tile on DVE
            # vs 1.73us PE consumption per tile) ---
            xps = {}

            def gen_xp(dt, k, split=1, eng=None):
                # xp[k] = xt[c-tile] * coef-broadcast[r],  k = r*CT + c-tile
                t = xppool.tile([P, N], bf16, name="xp", tag="xp")
                r, c = k // CT, k % CT
                fw = N // split
                for s in range(split):
                    fsl = slice(s * fw, (s + 1) * fw)
                    (eng or nc.vector).scalar_tensor_tensor(
                        t[:, fsl], xt_sb[c][:, fsl], 1.0, cb_sb[r][:, fsl], mult, mult
                    )
                xps[dt, k] = t

            # fp8 DoubleRow lhsT tiles: generated once, resident, scale 1/8
            # folded into the DVE op. Plane i of tile kk covers fused-k rows
            # (KB + 2*kk + i)*P .. +P  ==  (r=R-1, c-tiles CT-J+2*kk+i).
            xp8_sb = []
            for kk in range(JD):
                t = cpool.tile([P, 2, N], fp8, name=f"xp8_{kk}", tag=f"xp8_{kk}")
                xp8_sb.append(t)

            def gen_xp8(kk, i):
                c = (KB + 2 * kk + i) % CT
                r = (KB + 2 * kk + i) // CT
                nc.vector.scalar_tensor_tensor(
                    xp8_sb[kk][:, i, :], xt_sb[c], 0.125, cb_sb[r], mult, mult
                )

            def mm(m, k, dt, start, stop):
                if k < KB:
                    nc.tensor.matmul(
                        ps[m],
                        xps[dt, k][:, m * P : (m + 1) * P],
                        wt_sb[k, dt],
                        start=start,
                        stop=stop,
                    )
                else:
                    nc.tensor.matmul(
                        ps[m],
                        xp8_sb[k - KB][:, :, m * P : (m + 1) * P],
                        wt_sb[k, dt],
                        start=start,
                        stop=stop,
                        perf_mode=DR,
                    )

            # dt0 generations, all on vector (the STT opcode only exists on
            # the DVE): 1.22us/tile vs 1.73us PE consumption, pool-paced ~14
            # tiles ahead. The fp8 tiles (first used at dt0's tail, ~115us)
            # are generated around k=40 where the lookahead buffer absorbs
            # their ~5us cost without stalling the bf16 stream.
            for dt in range(DT):
                xps[dt, 0], xps[dt, 1] = xp01_sb[0], xp01_sb[1]
            for k in range(2, KB):
                if k == 40:
                    for kk in range(JD):
                        for i in range(2):
                            gen_xp8(kk, i)
                gen_xp(0, k)

            NTAIL = MTAIL + JD
            for dt in range(DT):
                dsl = slice(dt * DTILE, (dt + 1) * DTILE)
                for k in range(NK - NTAIL):
                    for m in range(MT):
                        mm(m, k, dt, k == 0, False)
                if dt == 0:
                    # emit dt1's first generations ahead of dt0's drains on
                    # the vector queue so the handoff never waits on a gen
                    for k in range(2, GEN_AHEAD):
                        gen_xp(1, k)
                # m-major tail: close each bank early so the DVE drain (+bias
                # add) and store pipeline against the remaining matmuls
                for m in range(MT):
                    for k in range(NK - NTAIL, NK):
                        mm(m, k, dt, False, k == NK - 1)
                    stage = stpool.tile([P, DTILE], f32, name="st", tag="st")
                    nc.vector.scalar_tensor_tensor(
                        stage, ps[m], 1.0, bias_sb[dt, m], mult, add
                    )
                    if dt < DT - 1:
                        # mid-kernel stores: gpsimd (its ~8us end-drain then
                        # overlaps compute, not the exit barrier)
                        nc.gpsimd.dma_start(out[m * P : (m + 1) * P, dsl], stage)
                    else:
                        splits = 2 if m >= MT - 2 else 1
                        engs = [nc.sync, nc.scalar]
                        rw = P // splits
                        for s in range(splits):
                            engs[(m + s) % 2].dma_start(
                                out[m * P + s * rw : m * P + (s + 1) * rw, dsl],
                                stage[s * rw : (s + 1) * rw, :],
                            )
                if dt == 0:
                    for k in range(GEN_AHEAD, KB):
                        gen_xp(1, k)
    nc.finalize()
    return nc


def _get_nc():
    if "nc" not in _CACHE:
        _CACHE["nc"] = _build_nc()
    return _CACHE["nc"]


def _prepare_in_maps(inputs):
    import ml_dtypes

    bf = ml_dtypes.bfloat16
    f8 = ml_dtypes.float8_e4m3fn
    f32 = np.float32
    input_ = np.asarray(inputs["input"], dtype=f32)
    weight = np.asarray(inputs["weight"], dtype=f32)
    bias = np.asarray(inputs["bias"], dtype=f32)
    coef = np.asarray(inputs["coef"], dtype=f32)

    wt_full = np.ascontiguousarray(weight.transpose(2, 1, 0)).reshape(K, D)
    wt = np.ascontiguousarray(wt_full[: KB * P]).astype(bf)
    biasnd = np.ascontiguousarray(coef @ bias.T).astype(bf)
    # coef rows broadcast across partitions: coefbc[r*P+p, n] = coef[n, r]
    coefbc = np.ascontiguousarray(
        np.broadcast_to(coef.T[:, None, :], (R, P, N)).reshape(R * P, N)
    ).astype(bf)

    shared = {"wt": wt, "biasnd": biasnd, "coefbc": coefbc}
    if J:
        w8 = (wt_full[KB * P :] * 8.0).astype(f8)  # [J*P, D]
        # [dt, kk, p, i, f] -> rows (dt*JD+kk)*P+p, cols i*DTILE+f
        w8r = w8.reshape(JD, 2, P, DT, DTILE)
        shared["wt8"] = np.ascontiguousarray(
            w8r.transpose(3, 0, 2, 1, 4).reshape(DT * JD * P, 2 * DTILE)
        )

    coefT_f32 = coef.T
    in_maps = []
    for b in range(B):
        xt_b = input_[b].T  # [C, N]
        xp01 = (coefT_f32[0][None, :] * xt_b[: 2 * P]).astype(bf)
        m = {
            "xt": np.ascontiguousarray(xt_b).astype(bf),
            "xp01": np.ascontiguousarray(xp01),
            **shared,
        }
        in_maps.append(m)
    return in_maps


def _install_ntff_hook_shim():
    """The agent image lacks antenv.axon_hooks; recreate it from the ctypes
    hook factory in trn_agent_boot so trace=True can capture NTFF profiles."""
    import types

    if "antenv.axon_hooks" in sys.modules:
        return
    try:
        from trn_agent_boot.trn_boot import _ntff_profile_via_ctypes

        hook = _ntff_profile_via_ctypes("/opt/axon/libaxon_pjrt.so")
        mod = types.ModuleType("antenv.axon_hooks")
        mod.get_axon_ntff_profile_hook = lambda: hook
        sys.modules["antenv.axon_hooks"] = mod
    except Exception as e:  # profiling is best-effort; execution still works
        print(f"ntff hook shim unavailable: {e}")


def _run(inputs, trace=False, **kwargs):
    from concourse.bass_utils import run_bass_kernel_spmd

    if trace:
        _install_ntff_hook_shim()
    in_maps = _prepare_in_maps(inputs)
    nc = _get_nc()
    res = run_bass_kernel_spmd(
        nc, in_maps, core_ids=list(range(N_CORES)), trace=trace, **kwargs
    )
    out = np.stack([r["out"] for r in res.results], axis=0)
    return out, res


def kernel(**inputs) -> np.ndarray:
    out, _ = _run(inputs)
    return out
